# revision 8
# baseline (speedup 1.0000x reference)
"""Trainium2 Bass kernel for 3x3 conv (stride 1, pad 1) + bias.

x [32, 64, 224, 224] f32, weight [128, 64, 3, 3] f32, bias [128] f32
-> out [32, 128, 224, 224] f32.

Data-parallel over 8 NeuronCores: core c computes samples [4c, 4c+4).

Per-core scheme (v4, fp8 DoubleRow + bf16 stores):
- Host splits x into x_hi = e4m3(x), x_lo = e4m3(x - x_hi) stacked on the
  channel dim: xhl [N, 128, 226, 226] e4m3 (padded). Partitions 0-63 hold
  x_hi, 64-127 x_lo — no SBUF duplication DMAs.
- Weights: w_hi = e4m3(w), w_lo = e4m3(w - w_hi). 18 K=128 tiles per
  2-row output block: 9 "hi" tiles ([w_hi;w_hi] . [x_hi;x_lo] = x*w_hi)
  and 9 "lo" tiles ([w_lo;w_lo] . [x_hi;x_lo] = x*w_lo with the tiny
  x_lo*w_lo term included). Out = conv(x_hi+x_lo, w_hi) + conv(., w_lo)
  to e4m3 precision of the operands; rel RMS vs fp32 ~1.3e-3.
- DoubleRow packs 2 tiles per matmul: 9 DR matmuls per block, paired so
  the t-dim stride is a real positive offset (no stride-0 broadcast).
  Measured DR rate: 2 K-tiles per ~448 PE cycles per matmul.
- Output stored as bf16 (+~1.1e-3 RMS) halving store traffic; host
  widens bf16->f32 with a shift (exact).
- Whole sample resident in SBUF (51 KB/partition), double buffered.
- Loop order: matmul index outer, PSUM bank inner (4 banks x 2 rows).
- ScalarE evacuates psum->SBUF fused with bias add; 8-row store tiles.
  Loads ride the ACT HWDGE ring; stores 3:1 on SP:ACT to balance bytes.
"""
import numpy as np
import ml_dtypes

import concourse.bass as bass
import concourse.mybir as mybir
import concourse.tile as tile
from concourse import bacc
from concourse.ap import AP
from concourse.bass_utils import run_bass_kernel_spmd
from concourse._compat import axon_active

N_CORES = 8
S = 4                 # samples per core
IC, OC, H, W = 64, 128, 224, 224
HP, WP = H + 2, W + 2  # padded input dims (226)
GROWS = 8             # output rows per store tile / psum group
NG = H // GROWS       # 28 groups per sample
BLK = 2               # output rows per psum bank
NBANK = GROWS // BLK  # 4 banks per group

F8 = mybir.dt.float8e4
F32 = mybir.dt.float32
BF16 = mybir.dt.bfloat16
E4 = ml_dtypes.float8_e4m3
BF = ml_dtypes.bfloat16

# DR matmul tap pairs: ((kind, kh, kw) for t=0, (kind, kh, kw) for t=1).
# Ordered so off(t1) > off(t0) where off = kh*WP + kw.
PAIRS = [
    (("hi", 0, 0), ("hi", 0, 1)),
    (("hi", 0, 2), ("hi", 1, 0)),
    (("hi", 1, 1), ("hi", 1, 2)),
    (("hi", 2, 0), ("hi", 2, 1)),
    (("lo", 0, 0), ("lo", 0, 1)),
    (("lo", 0, 2), ("lo", 1, 0)),
    (("lo", 1, 1), ("lo", 1, 2)),
    (("lo", 2, 1), ("lo", 2, 2)),
    (("lo", 2, 0), ("hi", 2, 2)),
]


def build_module(repeat=1):
    nc = bacc.Bacc("TRN2", target_bir_lowering=False, debug=not axon_active(),
                   enable_asserts=True, num_devices=N_CORES)
    xs = nc.dram_tensor("xs", [S, 2 * IC, HP, WP], F8, kind="ExternalInput").ap()
    wdr = nc.dram_tensor("wdr", [2 * IC, 9 * 2 * OC], F8, kind="ExternalInput").ap()
    bias = nc.dram_tensor("bias", [OC, 1], F32, kind="ExternalInput").ap()
    out = nc.dram_tensor("out", [S, OC, H, W], BF16, kind="ExternalOutput").ap()

    with tile.TileContext(nc) as tc:
        with tc.tile_pool(name="wp", bufs=1) as wp, \
             tc.tile_pool(name="xp", bufs=2) as xp, \
             tc.tile_pool(name="op", bufs=3) as op, \
             tc.tile_pool(name="pp", bufs=2, space="PSUM") as pp:
            wt = wp.tile([2 * IC, 9, 2, OC], F8)
            btile = wp.tile([OC, 1], F32)
            nc.sync.dma_start(out=wt, in_=wdr.rearrange(
                "p (i t m) -> p i t m", i=9, t=2))
            nc.sync.dma_start(out=btile, in_=bias)

            def compute():
                for s in range(S):
                    xt = xp.tile([2 * IC, HP * WP], F8, tag="x")
                    nc.scalar.dma_start(
                        out=xt, in_=xs[s].rearrange("c h w -> c (h w)"))
                    xr = xt.rearrange("p (r c) -> p r c", c=WP)
                    for g in range(NG):
                        ot = op.tile([OC, GROWS, W], BF16)
                        psums = [pp.tile([OC, BLK, W], F32, name=f"ps{bb}",
                                         tag=f"ps{bb}")
                                 for bb in range(NBANK)]
                        for i, ((_, kh0, kw0), (_, kh1, kw1)) in enumerate(PAIRS):
                            delta = (kh1 - kh0) * WP + (kw1 - kw0)
                            for bb in range(NBANK):
                                u = g * GROWS + bb * BLK
                                v = xr[:, u + kh0:u + kh0 + BLK, kw0:kw0 + W]
                                vap = [list(d) for d in v.ap]
                                rhs = AP(v.tensor, v.offset,
                                         [vap[0], [delta, 2], vap[1], vap[2]])
                                nc.tensor.matmul(
                                    psums[bb], wt[:, i], rhs,
                                    start=(i == 0), stop=(i == 8),
                                    perf_mode=mybir.MatmulPerfMode.DoubleRow,
                                    skip_group_check=True)
                        for bb in range(NBANK):
                            nc.scalar.activation(
                                ot[:, bb * BLK:(bb + 1) * BLK, :].rearrange(
                                    "p a b -> p (a b)"),
                                psums[bb].rearrange("p a b -> p (a b)"),
                                mybir.ActivationFunctionType.Identity,
                                bias=btile)
                        eng = nc.scalar if g % 4 == 3 else nc.sync
                        oh = g * GROWS
                        eng.dma_start(out=out[s, :, oh:oh + GROWS, :], in_=ot)

            if repeat == 1:
                compute()
            else:
                with tc.For_i(0, repeat, 1):
                    compute()

    nc.compile()
    return nc


def host_prep(weight, bias):
    w = np.asarray(weight, dtype=np.float32)          # [oc, ic, kh, kw]
    w_hi = w.astype(E4)
    w_lo = (w - w_hi.astype(np.float32)).astype(E4)
    wt_hi = np.transpose(w_hi, (1, 2, 3, 0))          # [ic, kh, kw, oc]
    wt_lo = np.transpose(w_lo, (1, 2, 3, 0))
    tap = {"hi": wt_hi, "lo": wt_lo}
    wdr = np.zeros((2 * IC, 9, 2, OC), E4)
    for i, (t0, t1) in enumerate(PAIRS):
        for t, (kind, kh, kw) in enumerate((t0, t1)):
            wdr[:IC, i, t, :] = tap[kind][:, kh, kw, :]
            wdr[IC:, i, t, :] = tap[kind][:, kh, kw, :]
    b = np.asarray(bias, dtype=np.float32).reshape(OC, 1)
    return wdr.reshape(2 * IC, 9 * 2 * OC), b


_E4_LUT = np.arange(65536, dtype=np.uint16).view(np.float16).astype(E4).view(np.uint8)


def _fast_e4m3(x32):
    """fp32 -> e4m3 via fp16 bit-pattern LUT (double rounding is harmless:
    the hi/lo pair self-corrects any hi-rounding difference)."""
    return _E4_LUT[x32.astype(np.float16).view(np.uint16)].view(E4)


def pad_x(x):
    """fp32 x [N, 64, 224, 224] -> padded hi/lo e4m3 [N, 128, 226, 226]."""
    x = np.asarray(x, dtype=np.float32)
    n = x.shape[0]
    xhl = np.zeros((n, 2 * IC, HP, WP), np.uint8)
    xi = _fast_e4m3(x)
    xhl[:, :IC, 1:1 + H, 1:1 + W] = xi.view(np.uint8)
    xlo = _fast_e4m3(x - xi.astype(np.float32))
    xhl[:, IC:, 1:1 + H, 1:1 + W] = xlo.view(np.uint8)
    return xhl.view(E4)


_module_cache = {}


def get_module(repeat=1):
    if repeat not in _module_cache:
        _module_cache[repeat] = build_module(repeat)
    return _module_cache[repeat]


def make_in_maps(x, weight, bias):
    wdr, b = host_prep(weight, bias)
    xhl = pad_x(x)
    return [{"xs": xhl[c * S:(c + 1) * S], "wdr": wdr, "bias": b}
            for c in range(N_CORES)]


def _widen_bf16(a):
    """bf16 -> f32 exactly via bit shift (fast, avoids ml_dtypes cast)."""
    u = np.ascontiguousarray(a).view(np.uint16).astype(np.uint32) << 16
    return u.view(np.float32)


def kernel(x, weight, bias):
    nc = get_module()
    in_maps = make_in_maps(x, weight, bias)
    res = run_bass_kernel_spmd(nc, in_maps, core_ids=list(range(N_CORES)))
    outs = [_widen_bf16(np.asarray(res.results[c]["out"])) for c in range(N_CORES)]
    return np.concatenate(outs, axis=0)


# revision 9
# speedup vs baseline: 1.3567x; 1.3567x over previous
"""Trainium2 Bass kernel for 3x3 conv (stride 1, pad 1) + bias.

x [32, 64, 224, 224] f32, weight [128, 64, 3, 3] f32, bias [128] f32
-> out [32, 128, 224, 224] f32.

Data-parallel over 8 NeuronCores: core c computes samples [4c, 4c+4).

Per-core scheme (v5, fp8 DoubleRow, 7 matmuls per 2-row block):
- The 3-term product out = x*w_hi + x_hi*w_lo (w_hi/w_lo = e4m3 split,
  x_hi/x_lo = e4m3 split of x; rel RMS vs fp32 ~1.3e-3) needs 13.5
  K=128 contraction tiles per block. Host prepares THREE input planes so
  taps can pair across partition halves:
    plane0: [x_hi ; x_lo]                 -> tiles x*w_hi[tap] (9 taps)
    plane1: [x_hi ; x_hi shifted up 1row] -> tiles x_hi*w_lo for tap
            pairs (kh,kw)+(kh+1,kw) in one K=128 tile
    plane2: [x_hi ; x_hi shifted (-2rows,+1col)] -> pairs (2,kw)+(0,kw+1)
  Tiles per block: 9 hi + 5 lo (A:{00,10} B:{11,21} C:{02,12} D:{20,01}
  E:{22}) = 14 -> 7 DoubleRow matmuls (2 tiles each, real t-strides).
- Measured DR rate ~448 PE cycles/matmul -> 7/9 of the v4 PE time.
- Output bf16 (+1.1e-3 RMS), host widens exactly. 56-row strips (58
  padded rows x 3 planes = 39.3KB/partition), double buffered.
- Loads split: planes 0-1 on ACT ring, plane 2 on SP; stores SP.
- ScalarE evacuates psum->SBUF fused with bias add; 8-row store tiles.
"""
import numpy as np
import ml_dtypes

import concourse.bass as bass
import concourse.mybir as mybir
import concourse.tile as tile
from concourse import bacc
from concourse.ap import AP
from concourse.bass_utils import run_bass_kernel_spmd
from concourse._compat import axon_active

N_CORES = 8
S = 4                 # samples per core
IC, OC, H, W = 64, 128, 224, 224
HP, WP = H + 2, W + 2  # padded input dims (226)
QROWS = 56            # output rows per strip
SROWS = QROWS + 2     # 58 padded rows per strip
NQ = H // QROWS       # 4 strips per sample
GROWS = 8             # output rows per store tile / psum group
NGS = QROWS // GROWS  # 7 groups per strip
BLK = 2               # output rows per psum bank
NBANK = GROWS // BLK  # 4 banks per group
PL = SROWS * WP       # plane stride inside a strip tile (13108)

F8 = mybir.dt.float8e4
F32 = mybir.dt.float32
BF16 = mybir.dt.bfloat16
E4 = ml_dtypes.float8_e4m3


def _off(kh, kw):
    return kh * WP + kw


# 14 K=128 tiles -> 7 DR matmuls. Each tile: (pos, wtop, wbot) where pos is
# the strip-local offset (plane*PL + tap offset) and wtop/wbot name the
# weight slice for partitions 0-63 / 64-127 ("hi"/"lo", kh, kw) or None.
def _tiles():
    t = {}
    for kh in range(3):
        for kw in range(3):
            t[f"hi{kh}{kw}"] = (0 * PL + _off(kh, kw),
                                ("hi", kh, kw), ("hi", kh, kw))
    t["A"] = (1 * PL + _off(0, 0), ("lo", 0, 0), ("lo", 1, 0))
    t["B"] = (1 * PL + _off(1, 1), ("lo", 1, 1), ("lo", 2, 1))
    t["C"] = (1 * PL + _off(0, 2), ("lo", 0, 2), ("lo", 1, 2))
    t["D"] = (2 * PL + _off(2, 0), ("lo", 2, 0), ("lo", 0, 1))
    t["E"] = (1 * PL + _off(2, 2), ("lo", 2, 2), None)
    return t


_T = _tiles()
# matmul i = (tile at t=0, tile at t=1); all pos(t1) > pos(t0)
MMS = [
    ("hi00", "hi01"),
    ("hi02", "hi10"),
    ("hi11", "hi12"),
    ("hi20", "hi21"),
    ("hi22", "A"),
    ("B", "D"),
    ("C", "E"),
]
NMM = len(MMS)


def build_module(repeat=1):
    nc = bacc.Bacc("TRN2", target_bir_lowering=False, debug=not axon_active(),
                   enable_asserts=True, num_devices=N_CORES)
    # xs[s, c, plane, r, w]
    xs = nc.dram_tensor("xs", [S, 2 * IC, 3, HP, WP], F8,
                        kind="ExternalInput").ap()
    wdr = nc.dram_tensor("wdr", [2 * IC, NMM * 2 * OC], F8,
                         kind="ExternalInput").ap()
    bias = nc.dram_tensor("bias", [OC, 1], F32, kind="ExternalInput").ap()
    out = nc.dram_tensor("out", [S, OC, H, W], BF16, kind="ExternalOutput").ap()

    with tile.TileContext(nc) as tc:
        with tc.tile_pool(name="wp", bufs=1) as wp, \
             tc.tile_pool(name="xp", bufs=2) as xp, \
             tc.tile_pool(name="op", bufs=3) as op, \
             tc.tile_pool(name="pp", bufs=2, space="PSUM") as pp:
            wt = wp.tile([2 * IC, NMM, 2, OC], F8)
            btile = wp.tile([OC, 1], F32)
            nc.sync.dma_start(out=wt, in_=wdr.rearrange(
                "p (i t m) -> p i t m", i=NMM, t=2))
            nc.sync.dma_start(out=btile, in_=bias)

            def compute():
                for s in range(S):
                    for q in range(NQ):
                        st = xp.tile([2 * IC, 3 * PL], F8, tag="x")
                        sv = st.rearrange("p (pl r c) -> p pl r c", pl=3, c=WP)
                        r0 = q * QROWS
                        nc.scalar.dma_start(
                            out=sv[:, 0:2],
                            in_=xs[s, :, 0:2, r0:r0 + SROWS, :])
                        nc.sync.dma_start(
                            out=sv[:, 2],
                            in_=xs[s, :, 2, r0:r0 + SROWS, :])
                        pap = list(st.ap[0])
                        for g in range(NGS):
                            ot = op.tile([OC, GROWS, W], BF16)
                            psums = [pp.tile([OC, BLK, W], F32, name=f"ps{bb}",
                                             tag=f"ps{bb}")
                                     for bb in range(NBANK)]
                            for i, (n0, n1) in enumerate(MMS):
                                p0, p1 = _T[n0][0], _T[n1][0]
                                delta = p1 - p0
                                for bb in range(NBANK):
                                    u = g * GROWS + bb * BLK
                                    rhs = AP(st.tensor, p0 + u * WP,
                                             [pap, [delta, 2], [WP, BLK], [1, W]])
                                    nc.tensor.matmul(
                                        psums[bb], wt[:, i], rhs,
                                        start=(i == 0), stop=(i == NMM - 1),
                                        perf_mode=mybir.MatmulPerfMode.DoubleRow,
                                        skip_group_check=True)
                            for bb in range(NBANK):
                                nc.scalar.activation(
                                    ot[:, bb * BLK:(bb + 1) * BLK, :].rearrange(
                                        "p a b -> p (a b)"),
                                    psums[bb].rearrange("p a b -> p (a b)"),
                                    mybir.ActivationFunctionType.Identity,
                                    bias=btile)
                            oh = q * QROWS + g * GROWS
                            nc.sync.dma_start(out=out[s, :, oh:oh + GROWS, :],
                                              in_=ot)

            if repeat == 1:
                compute()
            else:
                with tc.For_i(0, repeat, 1):
                    compute()

    nc.compile()
    return nc


def host_prep(weight, bias):
    w = np.asarray(weight, dtype=np.float32)          # [oc, ic, kh, kw]
    w_hi = w.astype(E4)
    w_lo = (w - w_hi.astype(np.float32)).astype(E4)
    tr = {"hi": np.transpose(w_hi, (1, 2, 3, 0)),     # [ic, kh, kw, oc]
          "lo": np.transpose(w_lo, (1, 2, 3, 0))}
    wdr = np.zeros((2 * IC, NMM, 2, OC), E4)
    for i, (n0, n1) in enumerate(MMS):
        for t, nm in enumerate((n0, n1)):
            _, wtop, wbot = _T[nm]
            kind, kh, kw = wtop
            wdr[:IC, i, t, :] = tr[kind][:, kh, kw, :]
            if wbot is not None:
                kind, kh, kw = wbot
                wdr[IC:, i, t, :] = tr[kind][:, kh, kw, :]
    b = np.asarray(bias, dtype=np.float32).reshape(OC, 1)
    return wdr.reshape(2 * IC, NMM * 2 * OC), b


_E4_LUT = np.arange(65536, dtype=np.uint16).view(np.float16).astype(E4).view(np.uint8)


def _fast_e4m3(x32):
    """fp32 -> e4m3 via fp16 bit-pattern LUT (double rounding is harmless:
    the hi/lo pair self-corrects any hi-rounding difference)."""
    return _E4_LUT[x32.astype(np.float16).view(np.uint16)].view(E4)


def pad_x(x):
    """fp32 x [N, 64, 224, 224] -> [N, 128, 3, 226, 226] e4m3 planes."""
    x = np.asarray(x, dtype=np.float32)
    n = x.shape[0]
    hi = np.zeros((n, IC, HP, WP), np.uint8)
    xi = _fast_e4m3(x)
    hi[:, :, 1:1 + H, 1:1 + W] = xi.view(np.uint8)
    lo = np.zeros((n, IC, HP, WP), np.uint8)
    lo[:, :, 1:1 + H, 1:1 + W] = _fast_e4m3(
        x - xi.astype(np.float32)).view(np.uint8)

    planes = np.zeros((n, 2 * IC, 3, HP, WP), np.uint8)
    for p in range(3):
        planes[:, :IC, p] = hi
    planes[:, IC:, 0] = lo
    # plane1 bottom: x_hi shifted up one row: b[r] = hi[r+1]
    planes[:, IC:, 1, :HP - 1, :] = hi[:, :, 1:, :]
    # plane2 bottom: b[r, c] = hi[r-2, c+1]
    planes[:, IC:, 2, 2:, :WP - 1] = hi[:, :, :HP - 2, 1:]
    return planes.view(E4)


_module_cache = {}


def get_module(repeat=1):
    if repeat not in _module_cache:
        _module_cache[repeat] = build_module(repeat)
    return _module_cache[repeat]


def make_in_maps(x, weight, bias):
    wdr, b = host_prep(weight, bias)
    xpl = pad_x(x)
    return [{"xs": xpl[c * S:(c + 1) * S], "wdr": wdr, "bias": b}
            for c in range(N_CORES)]


def _widen_bf16(a):
    """bf16 -> f32 exactly via bit shift (fast, avoids ml_dtypes cast)."""
    u = np.ascontiguousarray(a).view(np.uint16).astype(np.uint32) << 16
    return u.view(np.float32)


def kernel(x, weight, bias):
    nc = get_module()
    in_maps = make_in_maps(x, weight, bias)
    res = run_bass_kernel_spmd(nc, in_maps, core_ids=list(range(N_CORES)))
    outs = [_widen_bf16(np.asarray(res.results[c]["out"])) for c in range(N_CORES)]
    return np.concatenate(outs, axis=0)
